# revision 26
# baseline (speedup 1.0000x reference)
"""Trainium2 Bass kernel for the 5-stream dense transformer block
(nn_BWRPE_ater_19868518711491).

Strategy (8 NeuronCores, SPMD single program):
  - Token-sharded: core c owns tokens [128c, 128(c+1)) for all 10 (s,b)
    blocks.  All dense math runs in a TRANSPOSED layout (channels on
    partitions, tokens on the free dim) so no per-tile transposes are
    needed between matmuls; the x residual and the cross-stream adapter
    residuals are folded into the proj/fc2 PSUM accumulations with
    transpose / identity matmuls (no elementwise adds).
  - LN gamma/beta are folded into the following weights on the HOST;
    x ships pre-scaled by WS (LN is scale-invariant up to eps); weights
    ship pre-cast fp8 in their exact SBUF image.
  - Attention is sharded by (s,b,head) units in three sb-groups
    (sb 0-3 / 4-7 / 8-9); ownership is CONTIGUOUS (core d owns units
    [dK, (d+1)K) of the group) so qkv staging and o return staging
    batch into a few large DMAs.  Exchanged with AllToAll per group in
    each direction so collectives overlap compute in both phases.
  - softmax uses exp(x*scale) normalized by a matmul-accumulated
    denominator (ones column appended to V).
  - ACT table discipline: phases 1/2/3A only use Ln/Exp/Identity (one
    table set); all gelu (3B) runs after the last 3A so there is a
    single table switch in the whole kernel.
"""

import contextlib

import numpy as np

import concourse.bacc as bacc
import concourse.bass as bass
import concourse.mybir as mybir
import concourse.tile as tile
from concourse.bass_utils import run_bass_kernel_spmd
from concourse.masks import make_identity

# problem shapes (hardcoded per harness contract)
S, B, N, C, H, AD, HID = 5, 2, 1024, 768, 12, 8, 3072
D = C // H              # 64 head dim
SCALE = D ** -0.5
NCORES = 8
NT = N // NCORES        # 128 tokens per core per (s,b)
NSB = S * B             # 10 (s,b) blocks per core
NPAIR = S               # 5 stream-pairs (both batches of one stream)
P = 128
CC = C // P             # 6 contraction chunks of 128
KK = HID // P           # 24 hid chunks
NT2 = 2 * NT            # pair width (tokens of both batches)
EXP_BIAS = 0.0          # scores*scale in [-0.35, 0.35]; exp stays O(1)
EPS = 1e-5

F32 = mybir.dt.float32
BF16 = mybir.dt.bfloat16
FP8 = mybir.dt.float8e4
WS = 64.0               # fp8 weight pre-scale (undone in PSUM evacuation)
AF = mybir.ActivationFunctionType
ALU = mybir.AluOpType

# experiment knobs (fixed for submission)
FC2_FP8 = True          # fc2 weights+gelu acts in fp8 (DoubleRow) vs bf16
COLLECTIVES = True      # False = skip AllToAlls (timing ablation only)
QKV_EVAC_VECTOR = True  # qkv PSUM evacuation on DVE (False: ACT)
PH2_HALF = False        # phase2: 1-bank half-query score tiles + po x4
VB16_POOL = False       # vb16 cast copy on gpsimd instead of DVE
PH3_INTERLEAVE = True   # interleave A/B in phase3 (pays ACT table switches)

WEIGHT_NAMES = [
    "ln1_g", "ln1_b", "ln2_g", "ln2_b",
    "qkv_w", "qkv_b", "proj_w", "proj_b",
    "fc1_w", "fc1_b", "fc2_w", "fc2_b",
    "at_dw", "at_db", "at_mw", "at_mb", "at_uw", "at_ub",
    "a2_dw", "a2_db", "a2_mw", "a2_mb", "a2_uw", "a2_ub",
]

# ---- attention unit bookkeeping ----
# Unit u = sb*H + h.  Within group g (sb range [s0,s1)), core d owns the
# CONTIGUOUS slice units[d*K:(d+1)*K]; slot of u = (u - s0*H) % K.
GROUPS = [(0, 4), (4, 8), (8, 10)]          # sb ranges per exchange group
GRP_OF_SB = {sb: g for g, (s0, s1) in enumerate(GROUPS) for sb in range(s0, s1)}
UNITS = {}
SLOT = {}
LG = []
for g, (s0, s1) in enumerate(GROUPS):
    units = list(range(s0 * H, s1 * H))
    K = len(units) // NCORES
    LG.append(K)
    for d in range(NCORES):
        lst = units[d * K:(d + 1) * K]
        UNITS[(g, d)] = lst
        for i, u in enumerate(lst):
            SLOT[u] = (g, i)


def _sb_blocks(sb):
    """Owner blocks of one sb's heads.

    Yields (g, d, h0, K, subs): core d owns heads [h0, h0+K) of sb in
    group g.  Slot numbering within an owner is PARITY-MAJOR (all even
    heads by ascending c0, then all odd heads), so each sub-range
    (par, l0, c0_start, n) is a contiguous slot run AND a contiguous
    c0 run in one partition half -> single 3-dim DMA.
    """
    g = GRP_OF_SB[sb]
    s0 = GROUPS[g][0]
    K = LG[g]
    out = []
    for h0 in range(0, H, K):
        d = ((sb - s0) * H + h0) // K
        subs = []
        l0 = 0
        for par in (0, 1):
            hs = [h for h in range(h0, h0 + K) if h % 2 == par]
            if not hs:
                continue
            subs.append((par, l0, hs[0] // 2, len(hs)))
            l0 += len(hs)
        out.append((g, d, h0, K, subs))
    return out


# ---- fp8 weight image layout (columns per partition) ----
_off = 0
WOFF = {}
for _name, _w in [
    ("atd", 6 * AD), ("a2d", 6 * AD), ("atm", AD), ("a2m", AD),
    ("atu", C), ("a2u", C), ("vb_row", C), ("fc2b_row", C),
    ("qkv", CC * 3 * C),
    ("proj", CC * C), ("fc1", CC * HID),
    ("fc2", KK * C),
]:
    WOFF[_name] = _off
    _off += _w
WB_COLS = _off
SM_END = WOFF["qkv"]
QKV_END = WOFF["proj"]
WB2_END = WOFF["fc2"]
# f32 bias-column image
_off32 = 0
WOFF32 = {}
for _name, _w in [
    ("projb", CC), ("atub", CC), ("a2ub", CC), ("fc1b", KK),
    ("qkb", 18), ("atdb", 1), ("atmb", 1), ("a2db", 1), ("a2mb", 1),
]:
    WOFF32[_name] = _off32
    _off32 += _w
WF_COLS = _off32


def pack_weights(inp):
    """Fold LN gains into downstream weights; emit fp8 SBUF images."""
    f = {k: np.asarray(inp[k], dtype=np.float64) for k in WEIGHT_NAMES}
    g1, b1 = f["ln1_g"], f["ln1_b"]
    g2, b2 = f["ln2_g"], f["ln2_b"]
    qkv_w = g1[:, None] * f["qkv_w"]
    qkv_b = f["qkv_b"] + b1 @ f["qkv_w"]
    at_dw = g1[:, None] * f["at_dw"]
    at_db = f["at_db"] + b1 @ f["at_dw"]
    a2_dw = g1[:, None] * f["a2_dw"]
    a2_db = f["a2_db"] + b1 @ f["a2_dw"]
    fc1_w = g2[:, None] * f["fc1_w"]
    fc1_b = f["fc1_b"] + b2 @ f["fc1_w"]

    wb = np.zeros((P, WB_COLS), dtype=np.float32)
    sc = {"qkv": WS, "proj": 8.0, "fc1": WS, "fc2": WS,
          "atd": WS, "a2d": WS, "atm": WS, "a2m": WS, "atu": WS, "a2u": WS}

    def put_chunked(name, w):           # [K, M] -> [p, K//128, M]
        k, m = w.shape
        kc = k // P
        wb[:, WOFF[name]:WOFF[name] + kc * m] = sc[name] * (
            w.reshape(kc, P, m).transpose(1, 0, 2).reshape(P, kc * m))

    put_chunked("qkv", qkv_w)
    put_chunked("proj", f["proj_w"])
    put_chunked("fc1", fc1_w)
    kc2 = (WS * f["fc2_w"]).reshape(KK, P, C).transpose(1, 0, 2)
    wb2b = np.ascontiguousarray(kc2.reshape(P, KK * C)).astype(
        mybir.dt.np(FP8 if FC2_FP8 else BF16))
    put_chunked("atd", at_dw)
    put_chunked("a2d", a2_dw)
    wb[:AD, WOFF["atm"]:WOFF["atm"] + AD] = WS * f["at_mw"]
    wb[:AD, WOFF["a2m"]:WOFF["a2m"] + AD] = WS * f["a2_mw"]
    wb[:AD, WOFF["atu"]:WOFF["atu"] + C] = WS * f["at_uw"]
    wb[:AD, WOFF["a2u"]:WOFF["a2u"] + C] = WS * f["a2_uw"]
    wr = (WS * f["fc2_b"]).astype(mybir.dt.np(BF16)).reshape(1, C)

    wf = np.zeros((P, WF_COLS), dtype=np.float32)
    wf[:, WOFF32["projb"]:WOFF32["projb"] + CC] = WS * f["proj_b"].reshape(CC, P).T
    wf[:, WOFF32["atub"]:WOFF32["atub"] + CC] = WS * f["at_ub"].reshape(CC, P).T
    wf[:, WOFF32["a2ub"]:WOFF32["a2ub"] + CC] = WS * f["a2_ub"].reshape(CC, P).T
    wf[:, WOFF32["fc1b"]:WOFF32["fc1b"] + KK] = fc1_b.reshape(KK, P).T
    wf[:, WOFF32["qkb"]:WOFF32["qkb"] + 18] = qkv_b.reshape(18, P).T
    wf[:AD, WOFF32["atdb"]] = at_db
    wf[:AD, WOFF32["atmb"]] = f["at_mb"]
    wf[:AD, WOFF32["a2db"]] = a2_db
    wf[:AD, WOFF32["a2mb"]] = f["a2_mb"]
    return wb.astype(mybir.dt.np(FP8)), wb2b, wr, wf


def build_program(repeat: int = 1, debug: bool = False) -> bass.Bass:
    nc = bacc.Bacc(None, num_devices=NCORES)
    xs = nc.dram_tensor("xs", [NSB, NT, C], F32, kind="ExternalInput")
    wb = nc.dram_tensor("wb", [P, WB_COLS], FP8, kind="ExternalInput")
    wb2b = nc.dram_tensor("wb2b", [P, KK * C], FP8 if FC2_FP8 else BF16,
                          kind="ExternalInput")
    wr = nc.dram_tensor("wr", [1, C], BF16, kind="ExternalInput")
    wf = nc.dram_tensor("wf", [P, WF_COLS], F32, kind="ExternalInput")
    out = nc.dram_tensor("out", [NSB, NT, C], F32, kind="ExternalOutput")
    dbg = None
    if debug:
        dbg = {k: nc.dram_tensor(f"dbg_{k}", shp, dt, kind="ExternalOutput")
               for k, shp, dt in [
                   ("xnT", [P, CC, NT2], FP8), ("qkvT", [P, CC, 3, NT2], FP8),
                   ("a1T", [P, CC, NT2], FP8), ("oT3", [P, CC, NT2], FP8),
                   ("yT", [P, CC, NT2], F32), ("ynT", [P, CC, NT2], FP8),
                   ("a2T", [P, CC, NT2], FP8), ("ghT", [P, KK, NT2], FP8),
               ]}
    with tile.TileContext(nc) as tc:
        for _ in range(repeat):
            build_tile_kernel(tc, xs.ap(), wb.ap(), wb2b.ap(), wr.ap(), wf.ap(), out.ap(),
                              {k: v.ap() for k, v in dbg.items()} if dbg else None)
    nc.finalize()
    return nc


def build_tile_kernel(tc, xs, wbd, wb2bd, wrd, wfd, out, dbg=None):
    nc = tc.nc
    rg = [list(range(NCORES))]

    with contextlib.ExitStack() as ctx:
        sing = ctx.enter_context(tc.tile_pool(name="sing", bufs=1))
        keep = ctx.enter_context(tc.tile_pool(name="keep", bufs=1))
        wbig = ctx.enter_context(tc.tile_pool(name="wbig", bufs=1))
        dram = ctx.enter_context(tc.tile_pool(name="dram", bufs=1, space="DRAM"))

        # ---------------- constants ----------------
        ident_bf = sing.tile([P, P], BF16)
        make_identity(nc, ident_bf)
        ident_f8 = sing.tile([P, P], FP8)
        make_identity(nc, ident_f8)
        identF = sing.tile([P, P], F32)
        make_identity(nc, identF)
        nln_col = sing.tile([P, 1], F32)
        nc.vector.memset(nln_col, -float(np.log(WS)))
        eps_col = sing.tile([P, 1], F32)
        nc.vector.memset(eps_col, EPS)
        nexpb_col = sing.tile([P, 1], F32)
        nc.vector.memset(nexpb_col, -EXP_BIAS)
        ones_row = sing.tile([1, P], BF16)
        nc.vector.memset(ones_row, 1.0)
        oc_bf = sing.tile([P, 1], BF16)
        nc.vector.memset(oc_bf, 1.0 / C)
        oc_f32 = sing.tile([P, 1], F32)
        nc.vector.memset(oc_f32, 1.0 / C)

        # ---------------- weights (pre-packed images) ----------------
        w_sm = sing.tile([P, SM_END], FP8, name="w_sm")
        w_qkv = wbig.tile([P, QKV_END - SM_END], FP8, tag="big")
        w_big2 = sing.tile([P, WB2_END - QKV_END], FP8, name="w_big2")
        wf_t = sing.tile([P, WF_COLS], F32, name="wf")
        wr_t = sing.tile([1, C], BF16, name="wr_t")
        nc.gpsimd.dma_start(w_sm[:], wbd[:, :SM_END])
        nc.gpsimd.dma_start(wf_t[:], wfd[:])
        nc.gpsimd.dma_start(wr_t[:], wrd[:])
        nc.gpsimd.dma_start(w_qkv[:], wbd[:, SM_END:QKV_END])
        nc.gpsimd.dma_start(w_big2[:], wbd[:, QKV_END:WB2_END])

        qkv_v = w_qkv[:].rearrange("p (c m) -> p c m", m=3 * C)
        proj_v = w_big2[:, :CC * C].rearrange("p (c m) -> p c m", m=C)
        fc1_v = w_big2[:, CC * C:].rearrange("p (c m) -> p c m", m=HID)
        w_atd = w_sm[:, WOFF["atd"]:WOFF["atd"] + CC * AD].rearrange(
            "p (c m) -> p c m", m=AD)
        w_a2d = w_sm[:, WOFF["a2d"]:WOFF["a2d"] + CC * AD].rearrange(
            "p (c m) -> p c m", m=AD)
        w_atm = w_sm[:AD, WOFF["atm"]:WOFF["atm"] + AD]
        w_a2m = w_sm[:AD, WOFF["a2m"]:WOFF["a2m"] + AD]
        w_atu = w_sm[:AD, WOFF["atu"]:WOFF["atu"] + C]
        w_a2u = w_sm[:AD, WOFF["a2u"]:WOFF["a2u"] + C]
        fc2b_row = wr_t[0:1, :]

        def wfc(name, i, np_=P):        # f32 bias column
            return wf_t[:np_, WOFF32[name] + i:WOFF32[name] + i + 1]

        # ---------------- DRAM exchange buffers ----------------
        a1_in = [dram.tile([NCORES, D, LG[g], 3, NT], FP8, name=f"a1i{g}")
                 for g in range(len(GROUPS))]
        a1_out = [dram.tile([NCORES, D, LG[g], 3, NT], FP8, name=f"a1o{g}")
                  for g in range(len(GROUPS))]
        a2_in = [dram.tile([NCORES, LG[g], NT * D], FP8, name=f"a2i{g}")
                 for g in range(len(GROUPS))]
        a2_out = [dram.tile([NCORES, LG[g], NT * D], FP8, name=f"a2o{g}")
                  for g in range(len(GROUPS))]

        # ---------------- long-lived activation tiles ----------------
        a1T = [keep.tile([P, CC, NT2], FP8, name=f"a1T{j}")
               for j in range(NPAIR)]
        a2pool = ctx.enter_context(tc.tile_pool(name="a2p", bufs=5))
        ypool = ctx.enter_context(tc.tile_pool(name="yp", bufs=5))
        ynpool = ctx.enter_context(tc.tile_pool(name="ynp", bufs=5))
        a2T = [None] * NPAIR
        yT = [None] * NPAIR
        ynT = [None] * NPAIR

        # ---------------- PSUM pools (8 banks total) ----------------
        # ps_sc: 2 x 2-bank score accumulators; ps_a: 2 x 1-bank general;
        # ps_po: 2 x 1-bank attention-output accumulators
        ps_sc = ctx.enter_context(tc.tile_pool(name="ps_sc", bufs=2, space="PSUM"))
        ps_a = ctx.enter_context(tc.tile_pool(name="ps_a", bufs=2, space="PSUM"))
        ps_po = ctx.enter_context(tc.tile_pool(
            name="ps_po", bufs=4 if PH2_HALF else 2, space="PSUM"))

        def psA(shape, dtype=F32):
            return ps_a.tile(shape, dtype, tag="a", name="psa")

        p1 = ctx.enter_context(tc.tile_pool(name="p1", bufs=3))
        p2 = ctx.enter_context(tc.tile_pool(name="p2", bufs=3))
        p2e = ctx.enter_context(tc.tile_pool(name="p2e", bufs=3))
        p3 = ctx.enter_context(tc.tile_pool(name="p3", bufs=2))
        p3g = ctx.enter_context(tc.tile_pool(name="p3g", bufs=2))
        p2s = ctx.enter_context(tc.tile_pool(name="p2s", bufs=2))
        p1s = ctx.enter_context(tc.tile_pool(name="p1s", bufs=1))

        # ================= helpers =================
        def ln_natural(pool, x_f32, name):
            """natural-layout LN -> bf16 (x-mu)*rstd (gamma/beta folded).
            x ships WS-scaled; LN is scale-invariant (eps shift is
            negligible: eps/WS^2 vs eps on O(1) variance)."""
            stats = pool.tile([P, 3, 6], F32, tag=f"{name}_st")
            for i in range(3):
                nc.vector.bn_stats(stats[:, i, :], x_f32[:, i * 256:(i + 1) * 256])
            mv = pool.tile([P, 2], F32, tag=f"{name}_mv")
            nc.vector.bn_aggr(mv[:], stats[:])
            lnv = pool.tile([P, 1], F32, tag=f"{name}_lnv")
            nc.scalar.activation(lnv[:], mv[:, 1:2], AF.Ln, bias=eps_col, scale=1.0)
            rstd = pool.tile([P, 1], F32, tag=f"{name}_rs")
            nc.scalar.activation(rstd[:], lnv[:], AF.Exp, bias=0.0, scale=-0.5)
            xn = pool.tile([P, C], BF16, tag=f"{name}_xn")
            nc.vector.tensor_scalar(
                xn[:], x_f32, mv[:, 0:1], rstd[:], ALU.subtract, ALU.mult)
            return xn

        def adapter_T(pool, rhs_pair, wd, bd_col, wm, bm_col, wu, bu_base, dst,
                      evac_scalar=True):
            """transposed adapter: rhs_pair [128, CC, NT2] fp8 -> dst tile."""
            h1ps = psA([AD, NT2])
            for cc in range(CC):
                nc.tensor.matmul(h1ps[:], lhsT=wd[:, cc, :], rhs=rhs_pair[:, cc, :],
                                 start=(cc == 0), stop=(cc == CC - 1))
            h1 = pool.tile([AD, NT2], FP8, tag="ad_h1")
            nc.vector.tensor_scalar(h1[:], h1ps[:], 1.0 / WS, bd_col,
                                    ALU.mult, ALU.add)
            h2ps = psA([AD, NT2])
            nc.tensor.matmul(h2ps[:], lhsT=wm, rhs=h1[:], start=True, stop=True)
            h2 = pool.tile([AD, NT2], FP8, tag="ad_h2")
            nc.vector.tensor_scalar(h2[:], h2ps[:], 1.0 / WS, bm_col,
                                    ALU.mult, ALU.add)
            for m in range(CC):
                ups = psA([P, NT2])
                nc.tensor.matmul(ups[:], lhsT=wu[:, m * P:(m + 1) * P], rhs=h2[:],
                                 start=True, stop=True)
                if evac_scalar:
                    nc.scalar.activation(dst[:, m, :], ups[:], AF.Identity,
                                         bias=wfc(bu_base, m), scale=1.0)
                else:
                    nc.vector.tensor_scalar_add(dst[:, m, :], ups[:],
                                                wfc(bu_base, m))

        # ================= phase 1 =================
        for j in range(NPAIR):
            xnT = p1.tile([P, CC, NT2], FP8, tag="xnT")
            for jj in range(2):
                sb = 2 * j + jj
                x_t = p1.tile([P, C], F32, tag="x1")
                nc.scalar.dma_start(x_t[:], xs[sb])
                xn = ln_natural(p1, x_t[:], "ln1")
                for cc in range(CC):
                    pst = psA([P, P], BF16)
                    nc.tensor.transpose(pst[:], xn[:, cc * P:(cc + 1) * P],
                                        ident_bf[:])
                    if cc % 2 == 0:
                        nc.vector.tensor_copy(xnT[:, cc, jj * NT:(jj + 1) * NT],
                                              pst[:])
                    else:
                        nc.scalar.copy(xnT[:, cc, jj * NT:(jj + 1) * NT],
                                       pst[:])
            # full transposed qkv, stored c-major / t-inner ([P, CC, 3, NT2])
            # so one owner-block half (contiguous c0 run x all t) stages as
            # a single 3-dim DMA into parity-major contiguous slots.
            qkvT = p1s.tile([P, CC, 3, NT2], FP8, tag="qkvT")
            qkv_ct = qkvT[:].rearrange("p c t n -> p (c t) n")
            for c0 in range(CC):
                for t in range(3):
                    m = t * CC + c0
                    qps = psA([P, NT2])
                    for cp in range(CC // 2):
                        nc.tensor.matmul(
                            qps[:],
                            lhsT=qkv_v[:, 2 * cp:2 * cp + 2,
                                       m * P:(m + 1) * P],
                            rhs=xnT[:, 2 * cp:2 * cp + 2, :],
                            start=(cp == 0), stop=(cp == CC // 2 - 1),
                            perf_mode=mybir.MatmulPerfMode.DoubleRow)
                    if QKV_EVAC_VECTOR and t != 0:
                        nc.vector.tensor_scalar(
                            qkvT[:, c0, t, :], qps[:], 1.0 / WS,
                            wfc("qkb", m), ALU.mult, ALU.add)
                    else:
                        nc.scalar.activation(
                            qkvT[:, c0, t, :], qps[:], AF.Identity,
                            bias=wfc("qkb", m), scale=1.0 / WS)
                for jj in range(2):
                    sb = 2 * j + jj
                    for (g, d, h0, K, subs) in _sb_blocks(sb):
                        if (h0 + K - 1) // 2 != c0:
                            continue
                        for (par, l0, c0s, n) in subs:
                            nc.sync.dma_start(
                                a1_in[g][d, :, l0:l0 + n, :, :].rearrange(
                                    "dd l t n -> dd (l t) n"),
                                qkv_ct[par * D:(par + 1) * D,
                                       3 * c0s:3 * (c0s + n),
                                       jj * NT:(jj + 1) * NT])

            if COLLECTIVES and j in (1, 3, 4):
                g = {1: 0, 3: 1, 4: 2}[j]
                nc.gpsimd.collective_compute(
                    "AllToAll", ALU.bypass, replica_groups=rg,
                    ins=[a1_in[g][:].opt()], outs=[a1_out[g][:].opt()])
            adapter_T(p1, xnT, w_atd, wfc("atdb", 0, AD), w_atm,
                      wfc("atmb", 0, AD), w_atu, "atub", a1T[j])
            if dbg is not None and j == 0:
                nc.sync.dma_start(dbg["xnT"], xnT[:])
                nc.sync.dma_start(dbg["qkvT"], qkvT[:])
                nc.sync.dma_start(dbg["a1T"], a1T[0][:])

        # fc2 image loads into the slot qkv just freed
        w_fc2t = wbig.tile([P, KK * C], FP8 if FC2_FP8 else BF16, tag="big")
        nc.sync.dma_start(w_fc2t[:], wb2bd[:])
        fc2_v = w_fc2t[:].rearrange("p (c m) -> p c m", m=C)

        # ================= phase 2: attention units =================
        for g in range(len(GROUPS)):
            for l in range(LG[g]):
                qkv2 = p2.tile([D, NCORES, 3, NT], FP8, tag="qkv2")
                for t in range(3):
                    nc.sync.dma_start(
                        qkv2[:, :, t, :],
                        a1_out[g][:, :, l, t, :].rearrange(
                            "s dd n -> dd s n"))
                v_sb = p2s.tile([P, NCORES, D + 1], BF16, tag="v_sb")
                nc.vector.memset(v_sb[:, :, D:D + 1], 1.0)
                vb16 = p2.tile([D, N], BF16, tag="vb16")
                (nc.gpsimd if VB16_POOL else nc.vector).tensor_copy(
                    vb16[:].rearrange("dd (s n) -> dd s n", n=NT),
                    qkv2[:, :, 2, :])
                for mt in range(NCORES):
                    vps = psA([P, D], BF16)
                    nc.tensor.transpose(
                        vps[:], vb16[:, mt * NT:(mt + 1) * NT],
                        ident_bf[:D, :D])
                    nc.vector.tensor_copy(v_sb[:, mt, :D], vps[:])

                poa = ps_po.tile([P, 4 * 72], F32, tag="po")
                pob = ps_po.tile([P, 4 * 72], F32, tag="po")
                for mt in range(NCORES):
                    expT = p2e.tile([P, N], BF16, tag="expT")
                    if PH2_HALF:
                        for half in range(2):
                            ps = ps_sc.tile([P, 512], F32, tag="sc")
                            nc.tensor.matmul(
                                ps[:],
                                lhsT=qkv2[:, mt, 1, :],
                                rhs=qkv2[:, 4 * half:4 * half + 4, 0, :],
                                start=True, stop=True)
                            nc.scalar.activation(
                                expT[:, half * 512:(half + 1) * 512], ps[:],
                                AF.Exp, bias=nexpb_col, scale=SCALE)
                    else:
                        ps = ps_sc.tile([P, N], F32, tag="sc")
                        for half in range(2):
                            nc.tensor.matmul(
                                ps[:, half * 512:(half + 1) * 512],
                                lhsT=qkv2[:, mt, 1, :],
                                rhs=qkv2[:, 4 * half:4 * half + 4, 0, :],
                                start=True, stop=True)
                        nc.scalar.activation(
                            expT[:], ps[:], AF.Exp, bias=nexpb_col, scale=SCALE)
                    for nqb in range(NCORES):
                        po = poa if nqb < 4 else pob
                        o0 = (nqb % 4) * 72
                        nc.tensor.matmul(
                            po[:, o0:o0 + D + 1],
                            lhsT=expT[:, nqb * NT:(nqb + 1) * NT],
                            rhs=v_sb[:, mt, :],
                            start=(mt == 0), stop=(mt == NCORES - 1))
                oT = p2s.tile([D, N], FP8, tag="oT")
                for nqb in range(NCORES):
                    po = poa if nqb < 4 else pob
                    o0 = (nqb % 4) * 72
                    rden = p2.tile([P, 1], F32, tag="rden")
                    nc.vector.reciprocal(rden[:], po[:, o0 + D:o0 + D + 1])
                    o_bf = p2.tile([P, D], BF16, tag="o_bf")
                    nc.vector.tensor_scalar(o_bf[:], po[:, o0:o0 + D],
                                            rden[:], 8.0, ALU.mult, ALU.mult)
                    pst = psA([D, P], BF16)
                    nc.tensor.transpose(pst[:], o_bf[:], ident_bf[:])
                    nc.vector.tensor_copy(oT[:, nqb * NT:(nqb + 1) * NT], pst[:])
                nc.sync.dma_start(
                    a2_in[g][:, l].rearrange("s (dd n) -> dd s n", n=NT),
                    oT[:].rearrange("dd (s n) -> dd s n", n=NT))
            if COLLECTIVES:
                nc.gpsimd.collective_compute(
                    "AllToAll", ALU.bypass, replica_groups=rg,
                    ins=[a2_in[g][:].opt()], outs=[a2_out[g][:].opt()])

        # ================= phase 3 =================
        # A phases (proj + LN2 + adapter2) only use Ln/Exp/Identity ACT
        # functions; B phases (MLP) use Gelu.  Running all A before all B
        # costs exactly one ACT table switch.
        def phase3_A(j):
            oT = p3.tile([P, CC, NT2], FP8, tag="p3oT")
            for jj in range(2):
                sb = 2 * j + jj
                for (g, d, h0, K, subs) in _sb_blocks(sb):
                    for (par, l0, c0s, n) in subs:
                        nc.sync.dma_start(
                            oT[par * D:(par + 1) * D, c0s:c0s + n,
                               jj * NT:(jj + 1) * NT],
                            a2_out[g][d, l0:l0 + n].rearrange(
                                "l (dd n) -> dd l n", n=NT))
            x2 = []
            for jj in range(2):
                x_t = p3.tile([P, C], F32, tag="x3")
                nc.sync.dma_start(x_t[:], xs[2 * j + jj])  # WS-scaled on host
                x2.append(x_t)
            nbs = [nb for nb in (j - 1, j + 1) if 0 <= nb < NPAIR]
            yT[j] = ypool.tile([P, CC, NT2], F32, tag="yT", name="yT")
            for m in range(CC):
                pps = ps_sc.tile([P, NT2], F32, tag="sc")
                for cp in range(CC // 2):
                    nc.tensor.matmul(
                        pps[:],
                        lhsT=proj_v[:, 2 * cp:2 * cp + 2, m * P:(m + 1) * P],
                        rhs=oT[:, 2 * cp:2 * cp + 2, :],
                        start=(cp == 0), stop=False,
                        perf_mode=mybir.MatmulPerfMode.DoubleRow)
                for jj in range(2):
                    nc.tensor.matmul(
                        pps[:, jj * NT:(jj + 1) * NT],
                        lhsT=x2[jj][:, m * P:(m + 1) * P],
                        rhs=identF[:], is_transpose=True,
                        start=False, stop=False, skip_group_check=True)
                # cross-stream adapter residuals fold into the same PSUM
                for i, nb in enumerate(nbs):
                    nc.tensor.matmul(
                        pps[:], lhsT=ident_f8[:], rhs=a1T[nb][:, m, :],
                        start=False, stop=(i == len(nbs) - 1),
                        skip_group_check=True)
                nc.scalar.activation(yT[j][:, m, :], pps[:], AF.Identity,
                                     bias=wfc("projb", m), scale=1.0)
            yfl = yT[j][:].rearrange("p c n -> p (c n)")
            # transposed LN of y (stats via ones-column matmuls)
            sqb = p3g.tile([P, CC, NT2], BF16, tag="sqb")
            nc.vector.tensor_mul(sqb[:].rearrange("p c n -> p (c n)"), yfl, yfl)
            rows = psA([1, 512])
            for cc in range(CC):
                nc.tensor.matmul(rows[:, 0:256], lhsT=oc_f32[:],
                                 rhs=yT[j][:, cc, :],
                                 start=(cc == 0), stop=(cc == CC - 1))
            for cc in range(CC):
                nc.tensor.matmul(rows[:, 256:512], lhsT=oc_bf[:],
                                 rhs=sqb[:, cc, :],
                                 start=(cc == 0), stop=(cc == CC - 1))
            murow = p3g.tile([1, NT2], BF16, tag="murow")
            nc.scalar.copy(murow[:], rows[:, 0:256])
            mu2 = p3g.tile([1, NT2], F32, tag="mu2")
            nc.vector.tensor_mul(mu2[:], murow[:], murow[:])
            varr = p3g.tile([1, NT2], F32, tag="varr")
            nc.vector.tensor_sub(varr[:], rows[:, 256:512], mu2[:])
            lnr = p3g.tile([1, NT2], F32, tag="lnr")
            nc.scalar.activation(lnr[:], varr[:], AF.Ln, bias=eps_col[0:1],
                                 scale=1.0 / (WS * WS))
            mr_row = p3g.tile([1, 512], BF16, tag="mr_row")
            nc.vector.tensor_copy(mr_row[:, 0:256], murow[:])
            nc.scalar.activation(mr_row[:, 256:512], lnr[:], AF.Exp,
                                 bias=nln_col[0:1], scale=-0.5)
            bps = psA([P, 512])
            nc.tensor.matmul(bps[:], lhsT=ones_row[:], rhs=mr_row[:],
                             start=True, stop=True)
            mrB = p3g.tile([P, 512], BF16, tag="mrB")
            nc.vector.tensor_copy(mrB[:], bps[:])
            ynT[j] = ynpool.tile([P, CC, NT2], FP8, tag="ynT", name="ynT")
            for cc in range(CC):
                nc.vector.tensor_sub(sqb[:, cc, :], yT[j][:, cc, :],
                                     mrB[:, 0:256])
                nc.vector.tensor_mul(ynT[j][:, cc, :], sqb[:, cc, :],
                                     mrB[:, 256:512])
            a2T[j] = a2pool.tile([P, CC, NT2], FP8, tag="a2T", name="a2T")
            adapter_T(p3, ynT[j], w_a2d, wfc("a2db", 0, AD), w_a2m,
                      wfc("a2mb", 0, AD), w_a2u, "a2ub", a2T[j])
            if dbg is not None and j == 0:
                nc.sync.dma_start(dbg["oT3"], oT[:])
                nc.sync.dma_start(dbg["yT"], yT[0][:])
                nc.sync.dma_start(dbg["ynT"], ynT[0][:])
                nc.sync.dma_start(dbg["a2T"], a2T[0][:])

        def phase3_B(j):
            nbs = [nb for nb in (j - 1, j + 1) if 0 <= nb < NPAIR]
            ghT = p3g.tile([P, KK, NT2], FP8 if FC2_FP8 else BF16, tag="ghT")
            for kk in range(KK):
                fps = psA([P, NT2])
                for cp in range(CC // 2):
                    nc.tensor.matmul(
                        fps[:],
                        lhsT=fc1_v[:, 2 * cp:2 * cp + 2, kk * P:(kk + 1) * P],
                        rhs=ynT[j][:, 2 * cp:2 * cp + 2, :],
                        start=(cp == 0), stop=(cp == CC // 2 - 1),
                        perf_mode=mybir.MatmulPerfMode.DoubleRow)
                nc.scalar.activation(ghT[:, kk, :], fps[:], AF.Gelu,
                                     bias=wfc("fc1b", kk), scale=1.0 / WS)
            if dbg is not None and j == 0:
                nc.sync.dma_start(dbg["ghT"], ghT[:])
            for jj in range(2):
                sb = 2 * j + jj
                zfin = p3.tile([P, C], F32, tag="zfin")
                for lo, hi in ((0, 512), (512, 768)):
                    mps = ps_sc.tile([P, 512], F32, tag="sc")
                    if FC2_FP8:
                        for kk2 in range(KK // 2):
                            nc.tensor.matmul(
                                mps[:, :hi - lo],
                                lhsT=ghT[:, 2 * kk2:2 * kk2 + 2,
                                         jj * NT:(jj + 1) * NT],
                                rhs=fc2_v[:, 2 * kk2:2 * kk2 + 2, lo:hi],
                                start=(kk2 == 0), stop=False,
                                perf_mode=mybir.MatmulPerfMode.DoubleRow)
                    else:
                        for kk in range(KK):
                            nc.tensor.matmul(
                                mps[:, :hi - lo],
                                lhsT=ghT[:, kk, jj * NT:(jj + 1) * NT],
                                rhs=fc2_v[:, kk, lo:hi],
                                start=(kk == 0), stop=False)
                    nc.tensor.matmul(
                        mps[:, :hi - lo], lhsT=ones_row[:],
                        rhs=fc2b_row[:, lo:hi], start=False, stop=False)
                    ncc = (hi - lo) // P
                    for ci in range(ncc):
                        nc.tensor.matmul(
                            mps[:, ci * P:(ci + 1) * P],
                            lhsT=yT[j][:, lo // P + ci, jj * NT:(jj + 1) * NT],
                            rhs=identF[:], is_transpose=True,
                            start=False, stop=False,
                            skip_group_check=True)
                        # cross-stream a2 residuals: transpose-add via PE
                        for i, nb in enumerate(nbs):
                            last = (ci == ncc - 1) and (i == len(nbs) - 1)
                            nc.tensor.matmul(
                                mps[:, ci * P:(ci + 1) * P],
                                lhsT=a2T[nb][:, lo // P + ci,
                                             jj * NT:(jj + 1) * NT],
                                rhs=ident_f8[:],
                                start=False, stop=last,
                                skip_group_check=True)
                    nc.vector.tensor_scalar_mul(zfin[:, lo:hi],
                                                mps[:, :hi - lo], 1.0 / WS)
                nc.sync.dma_start(out[sb], zfin[:])

        if PH3_INTERLEAVE:
            phase3_A(0)
            phase3_A(1)
            phase3_B(0)
            phase3_A(2)
            phase3_B(1)
            phase3_A(3)
            phase3_B(2)
            phase3_A(4)
            phase3_B(3)
            phase3_B(4)
        else:
            for j in range(NPAIR):
                phase3_A(j)
            for j in range(NPAIR):
                phase3_B(j)


_CACHED_NC = None
_CACHED_W = None


def _get_program():
    global _CACHED_NC
    if _CACHED_NC is None:
        _CACHED_NC = build_program()
    return _CACHED_NC


def make_in_maps(inputs):
    """Per-core input dicts for run_bass_kernel_spmd (also used by test.py)."""
    global _CACHED_W
    xs_full = np.stack(
        [np.asarray(inputs[f"x{i}"], dtype=np.float32) for i in range(S)]
    )
    key = tuple(id(inputs[k]) for k in WEIGHT_NAMES)
    if _CACHED_W is None or _CACHED_W[0] != key:
        wbi, wb2i, wri, wfi = pack_weights(inputs)
        _CACHED_W = (key, wbi, wb2i, wri, wfi)
    _, wbi, wb2i, wri, wfi = _CACHED_W
    in_maps = []
    for c in range(NCORES):
        shard = np.ascontiguousarray(
            (xs_full[:, :, c * NT:(c + 1) * NT, :] * np.float32(WS))
            .reshape(NSB, NT, C))
        in_maps.append({"xs": shard, "wb": wbi, "wb2b": wb2i,
                        "wr": wri, "wf": wfi})
    return in_maps


def kernel(**inputs) -> np.ndarray:
    nc = _get_program()
    in_maps = make_in_maps(inputs)
    res = run_bass_kernel_spmd(nc, in_maps, core_ids=list(range(NCORES)))
    z = np.empty((S, B, N, C), dtype=np.float32)
    for c in range(NCORES):
        z[:, :, c * NT:(c + 1) * NT, :] = res.results[c]["out"].reshape(S, B, NT, C)
    return z
